# revision 41
# baseline (speedup 1.0000x reference)
"""Transformer block (BatchedPoincareBlock) Bass kernel for 8 TRN2 cores.

Sharding: Megatron head-split attention (2 heads/core, LN1 replicated, fused
LN->transpose->QKV), one AllToAll to re-shard token-wise, then
sequence-parallel AO + LN2 + MLP on 512 own tokens/core with full weights.
Softmax is max-free (scores bounded ~+-60 here): P = exp(s) unnormalized,
denominator via a ones column appended to the PV lhsT, divide at eviction.

v3: kt-major attention with multi-bank PSUM score tiles (96 wide exp
instructions instead of 160 narrow ones), causal mask via an accumulated
triangular matmul, fp16 on the x->h->Q/K->scores path (bf16 there injects
softmax-exponent noise; f32r matmuls are ~2-4x slower than bf16 on real
HW), bf16 on all linear paths (V/P/AO/MLP), LN gamma/beta folded into
weights host-side, A/B emission interleaved per batch sharing one
attention PSUM pool, AllToAll split by head-half to overlap the first
collective with the last attention head, early AO/MLP weight prefetch.

v4 (626us -> ~463us):
- deferred softmax normalization: evictions ship UNNORMALIZED attention
  rows (bf16 handles the dynamic range; scores bounded ~+-60) plus the
  per-(head,token) denominators, which ride as 2 extra rows per chunk of
  the second AllToAll.  The old recip->PE-broadcast->mul eviction chain
  stalled PE ~2.2us x16 via the shared score-PSUM tag; now eviction is
  two DVE copies.  Receive side rebuilds 1/den per source core with one
  selector matmul (selbc input) + tensor_mul before the AO matmuls.
- full wfcT (8MB bf16) made SBUF-resident during attention (k<4 queued
  on SP behind the x loads, k>=4 issued inside the collB wait window),
  so phase E has no FC weight traffic and no half-reload stall.
- score PSUM "ps" bufs 2->3 and the two live PV accumulators share 2
  tags (was 4), letting PE run 2 key-tiles ahead of the ACT exp.
- wpoT stream spread over 3 DMA queues; fine-grained per-(m,n) output
  writes; half-1 LN/copy work kept off ACT (Exp func-table thrash);
  startup const casts on Pool so DVE starts bn_stats immediately.

v5 (~463us -> ~407us in the same measurement regime):
- h0 denominators ride the FIRST AllToAll (both a2a payloads are 65-row
  chunks, row 64 = that half's den), so the whole h0 receive path -- den
  reciprocal, selector-broadcast, aol h0 normalize, and the h0-half of
  the first four AO accumulation groups (K-split over head-half dims,
  pao bufs=4 / ptr2 bufs=2) -- executes inside the collB wait window.
  The remaining groups and all h1 halves run normally after collB.
- reciprocals straight to bf16 (one DVE op per half, matches baseline
  precision); second wfcT half queued on SP behind the aol-h0 loads.
Failed experiments (documented so they are not retried): rsqrt via
ACT ln->exp (keeps the Exp func set but puts 2 serial ACT ops in the
LN chain on the contended engine: ~+60us); den DMAs reordered before
the aol loads (no sim change, HW-regime-confounded); q-major PV is
blocked on unmodeled LD_WEIGHTS cost; fp8 DoubleRow fails the max-abs
error budget.
"""
import sys
for p in ('/opt/trn_rl_repo', '/root/.axon_site/_ro/trn_rl_repo'):
    if p not in sys.path:
        sys.path.insert(0, p)
import numpy as np
import concourse.bass as bass
import concourse.mybir as mybir
import concourse.tile as tile
import concourse.bacc as bacc
from concourse.masks import make_identity

F32 = mybir.dt.float32
F32R = mybir.dt.float32r
BF16 = mybir.dt.bfloat16
F16 = mybir.dt.float16
AF = mybir.ActivationFunctionType

NC = 8
B, S, E, H = 2, 2048, 1024, 16
HD = E // H          # 64
DFF = 4 * E          # 4096
T = B * S            # 4096
TC = T // NC         # 512 own tokens
LN_EPS = 1e-5
NEG = -1e30


def np_dt(dt):
    if dt == BF16:
        import ml_dtypes
        return ml_dtypes.bfloat16
    if dt == F16:
        return np.float16
    return np.float32


class Cfg:
    def __init__(self, dt_qkv=F16, dt_att=F16, dt_ao=BF16, dt_mlp=BF16,
                 dt_x=F16, dt_p=BF16, reps=1, skip_trivial=True,
                 use_hw_gelu=True, no_comm=False, phases="ABCDE",
                 exp_cap=1024, split_coll=True, wp_queues=3,
                 dbg_fixed_wp=False):
        # Q/K path in fp16: bf16 there perturbs softmax exponents (rel err
        # amplified by exp -> 2.3e-2), f32r matmuls are ~2x slow on real HW
        # (despite the cost model).  fp16 keeps 1c/row PE speed with 8x
        # less quantization noise.  V/P/AO/MLP errors are linear -> bf16.
        self.dt_qkv = dt_qkv
        self.dt_att = dt_att
        self.dt_ao = dt_ao
        self.dt_mlp = dt_mlp
        self.dt_x = dt_x
        self.dt_p = dt_p
        self.reps = reps
        self.skip_trivial = skip_trivial
        self.use_hw_gelu = use_hw_gelu
        self.no_comm = no_comm
        self.phases = phases
        self.exp_cap = exp_cap
        self.split_coll = split_coll
        self.wp_queues = wp_queues
        self.dbg_fixed_wp = dbg_fixed_wp
        self.key = (str(dt_qkv), str(dt_att), str(dt_ao), str(dt_mlp),
                    str(dt_x), str(dt_p), reps, skip_trivial, use_hw_gelu,
                    no_comm, phases, exp_cap, split_coll, wp_queues,
                    dbg_fixed_wp)


def build_program(cfg):
    c = cfg
    nc = bacc.Bacc(None, target_bir_lowering=False)

    x_in = nc.dram_tensor("x", [T, E], cfg.dt_x, kind="ExternalInput")
    xown_in = nc.dram_tensor("x_own", [TC, E], F32, kind="ExternalInput")
    wqkvT_in = nc.dram_tensor("wqkvT", [E, 384], c.dt_qkv, kind="ExternalInput")
    bqkv_in = nc.dram_tensor("bqkv", [128, 3], F32, kind="ExternalInput")
    waoT_in = nc.dram_tensor("waoT", [E, E], c.dt_ao, kind="ExternalInput")
    bao_in = nc.dram_tensor("bao", [1, E], F32, kind="ExternalInput")
    wfcT_in = nc.dram_tensor("wfcT", [E, DFF], c.dt_mlp, kind="ExternalInput")
    bfc_in = nc.dram_tensor("bfc", [128, 32], F32, kind="ExternalInput")
    wpoT_in = nc.dram_tensor("wpoT", [DFF, E], c.dt_mlp, kind="ExternalInput")
    bpo_in = nc.dram_tensor("bpo", [1, E], F32, kind="ExternalInput")
    ln1g_in = nc.dram_tensor("ln1g", [128, 8], F32, kind="ExternalInput")
    ln1b_in = nc.dram_tensor("ln1b", [128, 8], F32, kind="ExternalInput")
    ln2g_in = nc.dram_tensor("ln2g", [128, 8], F32, kind="ExternalInput")
    ln2b_in = nc.dram_tensor("ln2b", [128, 8], F32, kind="ExternalInput")
    negtri_in = nc.dram_tensor("negtri", [128, 128], F32, kind="ExternalInput")
    selbc_in = nc.dram_tensor("selbc", [8, 512], c.dt_ao,
                              kind="ExternalInput")
    out_ext = nc.dram_tensor("out_own", [TC, E], F32, kind="ExternalOutput")

    skip = c.skip_trivial

    with tile.TileContext(nc) as tc:
        with (
            tc.tile_pool(name="consts", bufs=1) as consts,
            tc.tile_pool(name="wpool", bufs=1) as wpool,
            tc.tile_pool(name="stats", bufs=8) as stats_p,
            tc.tile_pool(name="small", bufs=4) as small,
            tc.tile_pool(name="dram", bufs=1, space="DRAM") as dram,
        ):
            ident_f32 = consts.tile([128, 128], F32, name="ident_f32",
                                    tag="ident_f32")
            make_identity(nc, ident_f32[:])
            ident = {}
            for dt in {c.dt_qkv, c.dt_att, c.dt_mlp, c.dt_p}:
                if dt == F32:
                    ident[dt] = ident_f32
                    continue
                idt = consts.tile([128, 128], dt, name=f"ident_{dt}",
                                  tag=f"ident_{dt}")
                nc.gpsimd.tensor_copy(idt[:], ident_f32[:])
                ident[dt] = idt
            negtri = consts.tile([128, 128], F32, name="negtri", tag="negtri")
            nc.sync.dma_start(negtri[:], negtri_in[:])
            negtri_a = consts.tile([128, 128], c.dt_p, name="negtri_a",
                                   tag="negtri_a")
            nc.gpsimd.tensor_copy(negtri_a[:], negtri[:])
            eps_t = consts.tile([128, 1], F32, name="eps_t", tag="eps_t")
            nc.vector.memset(eps_t[:], LN_EPS)
            ones64_f = consts.tile([1, 64], F32, name="ones64_f", tag="ones64_f")
            nc.vector.memset(ones64_f[:], 1.0)
            ones64 = consts.tile([1, 64], BF16, name="ones64", tag="ones64")
            nc.vector.tensor_copy(ones64[:], ones64_f[:])
            if not skip:
                ones128f = consts.tile([1, 128], F32, name="ones128f",
                                       tag="ones128f")
                nc.vector.memset(ones128f[:], 1.0)
                ones128 = consts.tile([1, 128], c.dt_mlp, name="ones128",
                                      tag="ones128")
                nc.vector.tensor_copy(ones128[:], ones128f[:])
            selb = consts.tile([8, 512], c.dt_ao, name="selb", tag="selb")
            nc.sync.dma_start(selb[:], selbc_in[:])
            vpc_f = consts.tile([128, 2], F32, name="vpc_f", tag="vpc_f")
            nc.vector.memset(vpc_f[:, 0:1], 1.0)
            nc.vector.memset(vpc_f[:, 1:2], 0.0)
            vpcols = consts.tile([128, 2], c.dt_p, name="vpcols", tag="vpcols")
            nc.gpsimd.tensor_copy(vpcols[:], vpc_f[:])
            ln = {}
            if not skip:
                bqkv = consts.tile([128, 3], F32, name="bqkv", tag="bqkv")
                bfc = consts.tile([128, 32], F32, name="bfc", tag="bfc")
                bpo = consts.tile([1, E], F32, name="bpo", tag="bpo")
                nc.sync.dma_start(bqkv[:], bqkv_in[:])
                nc.sync.dma_start(bfc[:], bfc_in[:])
                nc.sync.dma_start(bpo[:], bpo_in[:])
                bpo_r = consts.tile([1, E], c.dt_mlp, name="bpo_r", tag="bpo_r")
                nc.vector.tensor_copy(bpo_r[:], bpo[:])

            wqkvT = []
            for k in range(8):
                t = wpool.tile([128, 384], c.dt_qkv, name=f"wqkvT{k}",
                               tag=f"wqkvT{k}")
                eng = nc.sync if k % 2 == 0 else nc.scalar
                eng.dma_start(t[:], wqkvT_in[128 * k:128 * (k + 1), :])
                wqkvT.append(t)
            a2a_inA = dram.tile([E // 2 + 8, TC], c.dt_ao, name="a2a_inA",
                                tag="a2a_inA")
            a2a_outA = dram.tile([E // 2 + 8, TC], c.dt_ao, name="a2a_outA",
                                 tag="a2a_outA")
            a2a_inB = dram.tile([E // 2 + 8, TC], c.dt_ao, name="a2a_inB",
                                tag="a2a_inB")
            a2a_outB = dram.tile([E // 2 + 8, TC], c.dt_ao, name="a2a_outB",
                                 tag="a2a_outB")
            a2a_inF = dram.tile([E + 16, TC], c.dt_ao, name="a2a_inF",
                                tag="a2a_inF")
            a2a_outF = dram.tile([E + 16, TC], c.dt_ao, name="a2a_outF",
                                 tag="a2a_outF")

            def ln_stats(x_ap, tag):
                st = stats_p.tile([128, 2, 6], F32, name=f"st_{tag}", tag="st")
                xv = x_ap.rearrange("p (n f) -> p n f", n=2)
                nc.vector.bn_stats(st[:, 0, :], xv[:, 0, :])
                nc.vector.bn_stats(st[:, 1, :], xv[:, 1, :])
                mv = stats_p.tile([128, 2], F32, name=f"mv_{tag}", tag="mv")
                nc.vector.bn_aggr(mv[:], st[:])
                rs = stats_p.tile([128, 1], F32, name=f"rs_{tag}", tag="rs")
                nc.scalar.activation(rs[:], mv[:, 1:2], AF.Sqrt, bias=eps_t[:])
                nc.vector.reciprocal(rs[:], rs[:])
                nmr = stats_p.tile([128, 1], F32, name=f"nmr_{tag}", tag="nmr")
                nc.vector.tensor_scalar(nmr[:], mv[:, 0:1], rs[:], -1.0,
                                        op0=mybir.AluOpType.mult,
                                        op1=mybir.AluOpType.mult)
                return rs, nmr

            for rep in range(c.reps):
              with tc.tile_pool(name="sbW", bufs=1) as sbW:
                # early weight prefetch: emitted late (after A) so its DMAs
                # fill the attention window without delaying x loads
                waoT = []
                xo = []
                wf0 = []

                def emit_prefetch():
                    for k in range(8):
                        t = sbW.tile([128, E], c.dt_ao, name=f"waoT{k}",
                                     tag=f"waoT{k}")
                        nc.sync.dma_start(t[:],
                                          waoT_in[128 * k:128 * (k + 1), :])
                        waoT.append(t)
                    for m in range(4):
                        t = sbW.tile([128, E], F32, name=f"xo{m}", tag=f"xo{m}")
                        nc.sync.dma_start(t[:],
                                          xown_in[128 * m:128 * (m + 1), :])
                        xo.append(t)
                    if "E" in c.phases:
                        # full wfcT resident during attention: phase E then
                        # has no FC weight DMA (was a half-reload stall).
                        # k<4 load behind the x/waoT queue on SP; k>=4 are
                        # issued inside the collB wait window.
                        for k in range(8):
                            t = sbW.tile([128, DFF], c.dt_mlp, name=f"wf{k}",
                                         tag=f"wf{k}")
                            if k < 4 or "B" not in c.phases:
                                nc.sync.dma_start(
                                    t[:], wfcT_in[128 * k:128 * (k + 1), :])
                            wf0.append(t)

                with tc.tile_pool(name="bigAC", bufs=1) as big:
                    qkvT = [big.tile([128, T],
                                     c.dt_p if m == 2 else c.dt_att,
                                     name=f"qkvT{m}",
                                     tag=f"qkvT{m}") for m in range(3)]
                    attnT = big.tile([128, T], c.dt_ao, name="attnT", tag="attnT")

                    qT, kTt, vT = qkvT

                    def emit_A_quarters(nbs, half, sbA, psA, ptag, qtag,
                                        pbufs):
                        # half 0 runs alone (ACT mostly idle); half 1
                        # overlaps B(0)'s exp-heavy window.
                        on_act = (half == 0)
                        for nb in nbs:
                            xg = []
                            for g in range(2):
                                xt2 = sbA.tile([128, 2 * E], c.dt_x,
                                               name=f"x_{nb}_{g}", tag="xtn",
                                               bufs=3)
                                r0 = nb * 512 + g * 256
                                eng = nc.sync if g == 0 else nc.gpsimd
                                eng.dma_start(
                                    xt2[:].rearrange("p (t e) -> p t e", t=2),
                                    x_in[r0:r0 + 256, :].rearrange(
                                        "(t p) e -> p t e", p=128))
                                xg.append(xt2)
                            h4 = sbA.tile([128, 4 * E], c.dt_qkv,
                                           name=f"hn_{nb}", tag="hn",
                                           bufs=2)
                            for tt in range(4):
                                xt = xg[tt // 2][:, (tt % 2) * E:
                                                 (tt % 2 + 1) * E]
                                rs, nmr = ln_stats(xt, f"a{nb}{tt}")
                                dst = h4[:, tt * E:(tt + 1) * E]
                                if on_act and tt % 2 == 1:
                                    # half 0 runs before attention: ACT has
                                    # spare capacity, DVE is the A-phase
                                    # bottleneck
                                    nc.scalar.activation(
                                        dst, xt, AF.Identity,
                                        bias=nmr[:], scale=rs[:])
                                else:
                                    nc.vector.tensor_scalar(
                                        dst, xt, rs[:], nmr[:],
                                        op0=mybir.AluOpType.mult,
                                        op1=mybir.AluOpType.add)
                            hT = []
                            for e in range(8):
                                ptr = psA.tile([128, 512], c.dt_qkv,
                                               name=f"ptr_{nb}_{e}",
                                               tag=ptag, bufs=pbufs)
                                for tt in range(4):
                                    nc.tensor.transpose(
                                        ptr[:, tt * 128:(tt + 1) * 128],
                                        h4[:, tt * E + e * 128:
                                           tt * E + (e + 1) * 128],
                                        ident[c.dt_qkv][:])
                                ht = sbA.tile([128, 512], c.dt_qkv,
                                              name=f"hT_{nb}_{e}",
                                              tag=f"hT{e}", bufs=2)
                                if on_act and e % 2 == 0:
                                    nc.scalar.copy(ht[:], ptr[:])
                                else:
                                    # half 1 interleaves with B exps: any
                                    # ACT use there thrashes the Exp
                                    # function table
                                    nc.vector.tensor_copy(ht[:], ptr[:])
                                hT.append(ht)
                            for m in range(3):
                                pq = psA.tile([128, 512], F32,
                                              name=f"pq_{nb}_{m}",
                                              tag=qtag, bufs=pbufs)
                                for k in range(8):
                                    nc.tensor.matmul(
                                        pq[:],
                                        wqkvT[k][:, m * 128:(m + 1) * 128],
                                        hT[k][:], start=(k == 0),
                                        stop=(k == 7))
                                dst = qkvT[m][:, nb * 512:(nb + 1) * 512]
                                if skip:
                                    if on_act:
                                        nc.scalar.copy(dst, pq[:])
                                    else:
                                        nc.vector.tensor_copy(dst, pq[:])
                                else:
                                    nc.scalar.activation(dst, pq[:],
                                                         AF.Identity,
                                                         bias=bqkv[:, m:m + 1])

                    def emit_B_pre(b, sbB, psB, kts=range(16)):
                        c0 = b * S
                        # V transposes for this batch (psum via shared tag)
                        vh = {}
                        for kt in kts:
                            ptv = psB.tile([128, 128], c.dt_p,
                                           name=f"ptv_{b}_{kt}", tag="ps",
                                           bufs=3)
                            nc.tensor.transpose(
                                ptv[:],
                                vT[:, c0 + kt * 128:c0 + (kt + 1) * 128],
                                ident[c.dt_p][:])
                            for h in range(2):
                                vt = sbB.tile([128, 66], c.dt_p,
                                              name=f"vp_{b}_{h}_{kt}",
                                              tag=f"vp{h}_{kt}")
                                if b == 0:
                                    # batch 0 overlaps the DVE-bound LN
                                    # window; ACT has headroom there
                                    nc.scalar.copy(
                                        vt[:, 0:64],
                                        ptv[:, 64 * h:64 * h + 64])
                                else:
                                    nc.vector.tensor_copy(
                                        vt[:, 0:64],
                                        ptv[:, 64 * h:64 * h + 64])
                                nc.vector.tensor_copy(vt[:, 64:66],
                                                      vpcols[:])
                                vh[(h, kt)] = vt
                        return vh

                    def emit_B_head(b, hset, vh, sbB, psB, passes=(0, 1)):
                        c0 = b * S
                        if True:
                            # two query-half passes per head: pass 0
                            # (q<1024, key tiles 0..7) needs only the
                            # first two LN/QKV token blocks, so attention
                            # starts halfway through phase A.  Per-group
                            # flag patterns match the verified single-pass.
                            for h, (p_lo, p_hi, nkt, Jset) in [
                                    (hh, pp) for hh in hset
                                    for pi, pp in enumerate(
                                        ((0, S // 2, 8, (0, 1)),
                                         (S // 2, S, 16, (2, 3))))
                                    if pi in passes]:
                                rq = 64 * h
                                ppv = {J: psB.tile([66, 512], F32,
                                                name=f"ppv_{b}_{h}_{J}",
                                                tag=f"ppv{J % 2}", bufs=1)
                                       for J in Jset}
                                for kt in range(nkt):
                                    qb = 128 * kt
                                    base = max(qb, p_lo)
                                    qw = p_hi - base
                                    nch = (qw + c.exp_cap - 1) // c.exp_cap
                                    P = sbB.tile([128, qw], c.dt_p,
                                                 name=f"P_{b}_{h}_{p_lo}_{kt}",
                                                 tag="P", bufs=4)
                                    for ch in range(nch):
                                        w0 = ch * c.exp_cap
                                        w1 = min(qw, w0 + c.exp_cap)
                                        ps = psB.tile([128, c.exp_cap], F32,
                                                      name=f"ps_{b}_{h}_{p_lo}_{kt}_{ch}",
                                                      tag="ps", bufs=3)
                                        for s0 in range(w0, w1, 512):
                                            s1 = min(w1, s0 + 512)
                                            diag = (s0 == 0
                                                    and base == qb)
                                            nc.tensor.matmul(
                                                ps[:, s0 - w0:s1 - w0],
                                                kTt[rq:rq + 64,
                                                    c0 + qb:c0 + qb + 128],
                                                qT[rq:rq + 64,
                                                   c0 + base + s0:
                                                   c0 + base + s1],
                                                start=True, stop=not diag)
                                            if diag:
                                                # causal mask via accumulated
                                                # triangular matmul
                                                nc.tensor.matmul(
                                                    ps[:, 0:128],
                                                    ident[c.dt_p][:],
                                                    negtri_a[:],
                                                    start=False, stop=True,
                                                    skip_group_check=True)
                                        nc.scalar.activation(
                                            P[:, w0:w1], ps[:, 0:w1 - w0],
                                            AF.Exp)
                                    for J in Jset:
                                        if J < kt // 4:
                                            continue
                                        off = max(0, qb - 512 * J)
                                        l0 = 512 * J + off - base
                                        nc.tensor.matmul(
                                            ppv[J][:, off:512],
                                            vh[(h, kt)][:],
                                            P[:, l0:l0 + 512 - off],
                                            start=(kt == 0),
                                            stop=(kt == 4 * J + 3),
                                            skip_group_check=True)
                                    if kt % 4 == 3 and kt // 4 in Jset:
                                        # deferred normalization: ship
                                        # unnormalized rows; den rows go to
                                        # DRAM for a tiny side-collective
                                        # (no PE / psum chain here)
                                        J = kt // 4
                                        nc.vector.tensor_copy(
                                            attnT[rq:rq + 64,
                                                  c0 + J * 512:c0 + (J + 1) * 512],
                                            ppv[J][0:64, :])
                                        jj = 4 * b + J
                                        dsb = sbB.tile([1, 512], c.dt_ao,
                                                       name=f"dsb_{b}_{h}_{J}",
                                                       tag="dsb", bufs=2)
                                        nc.vector.tensor_copy(dsb[:],
                                                              ppv[J][64:65, :])
                                        if c.split_coll:
                                            dbuf = a2a_inA if h == 0 else a2a_inB
                                            dden = dbuf[65 * jj + 64:
                                                        65 * jj + 65, :]
                                        else:
                                            dden = a2a_inF[130 * jj + 128 + h:
                                                           130 * jj + 129 + h, :]
                                        nc.sync.dma_start(dden, dsb[:])

                    # ---- Phases A+B interleaved per batch ----
                    # A(batch0) in its own psum pool; then one shared
                    # attention psum pool (tags: ps bufs=2 -> 4 banks,
                    # ppv0-3 -> 4 banks) hosts B(0), A(batch1)'s
                    # transposes/QKV (via the shared "ps" tag), and B(1),
                    # so PE work interleaves by slot rotation.  The
                    # AllToAll is split by head-half: the h=0 collective
                    # launches while h=1 attention still computes.
                    def stage_half(h, dst):
                        r = 64 * h
                        for j in range(8):
                            eng = nc.sync if j % 2 == 0 else nc.gpsimd
                            eng.dma_start(dst[65 * j:65 * j + 64, :],
                                          attnT[r:r + 64,
                                                512 * j:512 * (j + 1)])

                    def coll(ins, outs):
                        if c.no_comm:
                            nc.sync.dma_start(outs[:], ins[:])
                        else:
                            nc.gpsimd.collective_compute(
                                "AllToAll", mybir.AluOpType.bypass,
                                replica_groups=[list(range(NC))],
                                ins=[ins.opt()], outs=[outs.opt()],
                            )

                    if "B" in c.phases:
                        # one PSUM pool spans LN/QKV and attention via the
                        # shared "ps" tag, so attention pass 0 can begin as
                        # soon as its two token blocks of Q/K/V exist --
                        # no pool barrier between the phases.
                        with (
                            tc.tile_pool(name="sbB", bufs=1) as sbB,
                            tc.tile_pool(name="sbA0", bufs=1) as sbA0,
                            tc.tile_pool(name="psAtt", bufs=1,
                                         space="PSUM") as psAtt,
                        ):
                            emit_A_quarters(range(0, 4), 0, sbA0, psAtt,
                                            "ps", "ps", 3)
                            # V transposes emitted per key-half so pass 0
                            # is not queue-gated behind transposes that
                            # need the later QKV token blocks
                            vh0 = emit_B_pre(0, sbB, psAtt, range(0, 8))
                            emit_B_head(0, (0, 1), vh0, sbB, psAtt,
                                        passes=(0,))
                            vh0.update(emit_B_pre(0, sbB, psAtt,
                                                  range(8, 16)))
                            emit_B_head(0, (0, 1), vh0, sbB, psAtt,
                                        passes=(1,))
                            emit_A_quarters(range(4, 8), 1, sbA0, psAtt,
                                            "ps", "ps", 3)
                            emit_prefetch()
                            vh1 = emit_B_pre(1, sbB, psAtt, range(0, 8))
                            emit_B_head(1, (0,), vh1, sbB, psAtt,
                                        passes=(0,))
                            vh1.update(emit_B_pre(1, sbB, psAtt,
                                                  range(8, 16)))
                            emit_B_head(1, (0,), vh1, sbB, psAtt,
                                        passes=(1,))
                            if "C" in c.phases and c.split_coll:
                                stage_half(0, a2a_inA)
                                coll(a2a_inA, a2a_outA)
                            emit_B_head(1, (1,), vh1, sbB, psAtt)
                            if "C" in c.phases and c.split_coll:
                                stage_half(1, a2a_inB)
                                coll(a2a_inB, a2a_outB)
                            if "C" in c.phases and not c.split_coll:
                                for j in range(8):
                                    eng = (nc.sync if j % 2 == 0
                                           else nc.gpsimd)
                                    eng.dma_start(
                                        a2a_inF[130 * j:130 * j + 128, :],
                                        attnT[:, 512 * j:512 * (j + 1)])
                                coll(a2a_inF, a2a_outF)
                    else:
                        with (
                            tc.tile_pool(name="sbA1", bufs=1) as sbA1,
                            tc.tile_pool(name="psA1", bufs=1,
                                         space="PSUM") as psA1,
                        ):
                            emit_A_quarters(range(0, 8), 0, sbA1, psA1,
                                            "ptr", "pq", 3)
                        emit_prefetch()

                # ============ Phases D-E (own tokens) ============
                with tc.tile_pool(name="sbDE", bufs=1) as sbD:
                    if "D" not in c.phases:
                        continue
                    aol = []
                    for k in range(8):
                        t = sbD.tile([128, TC], c.dt_ao, name=f"aol{k}",
                                     tag=f"aol{k}")
                        if c.split_coll:
                            # rows 0:64 = h0 of core k, 64:128 = h1
                            nc.sync.dma_start(t[0:64, :],
                                              a2a_outA[65 * k:65 * k + 64, :])
                            nc.gpsimd.dma_start(
                                t[64:128, :],
                                a2a_outB[65 * k:65 * k + 64, :])
                        else:
                            eng = nc.sync if k % 2 == 0 else nc.gpsimd
                            eng.dma_start(t[:],
                                          a2a_outF[130 * k:130 * k + 128, :])
                        aol.append(t)
                    denLA = sbD.tile([8, TC], c.dt_ao, name="denLA",
                                     tag="denLA")
                    denLB = sbD.tile([8, TC], c.dt_ao, name="denLB",
                                     tag="denLB")
                    srcA = a2a_outA if c.split_coll else a2a_outF
                    srcB = a2a_outB if c.split_coll else a2a_outF
                    rw = 65 if c.split_coll else 130
                    d0 = 64 if c.split_coll else 128
                    d1 = 64 if c.split_coll else 129
                    nc.sync.dma_start(
                        denLA[:].rearrange("j (o t) -> j o t", o=1),
                        srcA[:].rearrange("(j r) t -> j r t",
                                          r=rw)[:, d0:d0 + 1, :])
                    nc.gpsimd.dma_start(
                        denLB[:].rearrange("j (o t) -> j o t", o=1),
                        srcB[:].rearrange("(j r) t -> j r t",
                                          r=rw)[:, d1:d1 + 1, :])
                    if "E" in c.phases and "B" in c.phases:
                        # second half of wfcT transfers inside the collB
                        # wait, behind the aol-h0/denLA loads
                        for k in range(4, 8):
                            nc.sync.dma_start(
                                wf0[k][:],
                                wfcT_in[128 * k:128 * (k + 1), :])
                    with nc.allow_low_precision(reason="softmax recip"):
                        recbiA = sbD.tile([8, TC], c.dt_ao, name="recbiA",
                                          tag="recbiA")
                        nc.vector.reciprocal(recbiA[:], denLA[:])
                        recbiB = sbD.tile([8, TC], c.dt_ao, name="recbiB",
                                          tag="recbiB")
                        nc.vector.reciprocal(recbiB[:], denLB[:])

                    # ---- Phase D: AO + residual + LN2 + h2T ----
                    x2 = []
                    h2n = []
                    with tc.tile_pool(name="psD", bufs=1, space="PSUM") as psD:
                        # h0 normalize + h0-half AO partials run inside the
                        # collB wait (depend only on collA)
                        for k in range(8):
                            rbp0 = psD.tile([64, TC], F32, name=f"rbp0_{k}",
                                            tag="recb", bufs=2)
                            nc.tensor.matmul(rbp0[:],
                                             selb[:, 64 * k:64 * (k + 1)],
                                             recbiA[:], start=True, stop=True)
                            rbs0 = sbD.tile([64, TC], c.dt_ao,
                                            name=f"rbs0_{k}",
                                            tag="rbs0", bufs=2)
                            nc.scalar.copy(rbs0[:], rbp0[:])
                            nc.vector.tensor_mul(aol[k][0:64, :],
                                                 aol[k][0:64, :], rbs0[:])
                        split_groups = [(0, 0), (0, 1), (1, 0), (1, 1)]
                        pa_split = {}
                        for (m, n) in split_groups:
                            pa = psD.tile([128, 512], F32,
                                          name=f"pao_{m}_{n}",
                                          tag="pao", bufs=4)
                            for k in range(8):
                                nc.tensor.matmul(
                                    pa[:],
                                    aol[k][0:64, m * 128:(m + 1) * 128],
                                    waoT[k][0:64, n * 512:(n + 1) * 512],
                                    start=(k == 0), stop=False,
                                    skip_group_check=True)
                            pa_split[(m, n)] = pa
                        # h1 side (blocks on collB)
                        for k in range(8):
                            rbp1 = psD.tile([128, TC], F32, name=f"rbp1_{k}",
                                            tag="recb", bufs=2)
                            nc.tensor.matmul(rbp1[64:128, :],
                                             selb[:, 64 * k:64 * (k + 1)],
                                             recbiB[:], start=True, stop=True)
                            rbs1 = sbD.tile([128, TC], c.dt_ao,
                                            name=f"rbs1_{k}",
                                            tag="rbs1", bufs=2)
                            nc.scalar.copy(rbs1[64:128, :], rbp1[64:128, :])
                            nc.vector.tensor_mul(aol[k][64:128, :],
                                                 aol[k][64:128, :],
                                                 rbs1[64:128, :])
                        for m in range(4):
                            x2m = sbD.tile([128, E], F32, name=f"x2_{m}",
                                           tag=f"x2{m}")
                            for n in range(2):
                                if (m, n) in pa_split:
                                    pa = pa_split[(m, n)]
                                    for k in range(8):
                                        nc.tensor.matmul(
                                            pa[:],
                                            aol[k][64:128,
                                                   m * 128:(m + 1) * 128],
                                            waoT[k][64:128,
                                                    n * 512:(n + 1) * 512],
                                            start=False, stop=(k == 7),
                                            skip_group_check=True)
                                else:
                                    pa = psD.tile([128, 512], F32,
                                                  name=f"pao_{m}_{n}",
                                                  tag="pao", bufs=4)
                                    for k in range(8):
                                        nc.tensor.matmul(
                                            pa[:],
                                            aol[k][:, m * 128:(m + 1) * 128],
                                            waoT[k][:, n * 512:(n + 1) * 512],
                                            start=(k == 0), stop=(k == 7))
                                nc.vector.tensor_add(
                                    x2m[:, n * 512:(n + 1) * 512], pa[:],
                                    xo[m][:, n * 512:(n + 1) * 512])

                            x2.append(x2m)
                            rs, nmr = ln_stats(x2m[:], f"d{m}")
                            h = sbD.tile([128, E], c.dt_mlp, name=f"h2n_{m}",
                                         tag=f"h2n{m}")
                            nc.scalar.activation(h[:], x2m[:], AF.Identity,
                                                 bias=nmr[:], scale=rs[:])
                            h2n.append(h)
                        h2T = []
                        for e in range(8):
                            ptr = psD.tile([128, 512], c.dt_mlp, name=f"ptr2_{e}",
                                           tag="ptr2", bufs=2)
                            for tt in range(4):
                                nc.tensor.transpose(
                                    ptr[:, tt * 128:(tt + 1) * 128],
                                    h2n[tt][:, e * 128:(e + 1) * 128],
                                    ident[c.dt_mlp][:])
                            ht = sbD.tile([128, 512], c.dt_mlp, name=f"h2T_{e}",
                                          tag=f"h2T{e}")
                            nc.vector.tensor_copy(ht[:], ptr[:])
                            h2T.append(ht)

                    # ---- Phase E: MLP ----
                    if "E" not in c.phases:
                        continue
                    gT = [sbD.tile([128, TC], c.dt_mlp, name=f"gT{m}",
                                   tag=f"gT{m}") for m in range(32)]
                    with tc.tile_pool(name="psE", bufs=1, space="PSUM") as psE:
                        for m in range(32):
                            pf = psE.tile([128, 512], F32, name=f"pf_{m}",
                                          tag="pf", bufs=4)
                            for k in range(8):
                                nc.tensor.matmul(
                                    pf[:], wf0[k][:, m * 128:(m + 1) * 128],
                                    h2T[k][:], start=(k == 0), stop=(k == 7))
                            bias_ap = None if skip else bfc[:, m:m + 1]
                            if c.use_hw_gelu:
                                if bias_ap is None:
                                    nc.scalar.activation(gT[m][:], pf[:],
                                                         AF.Gelu)
                                else:
                                    nc.scalar.activation(gT[m][:], pf[:],
                                                         AF.Gelu, bias=bias_ap)
                            else:
                                emit_tanh_gelu(nc, small, gT[m], pf,
                                               bias_ap, m)

                        ppo_t = [psE.tile([128, 512], F32, name=f"ppo_{m}",
                                          tag=f"ppo{m}", bufs=1) for m in range(4)]
                        outb = [sbD.tile([128, E], F32, name=f"outb{m}",
                                         tag=f"outb{m}") for m in range(4)]
                        wp_engs = [nc.sync, nc.gpsimd, nc.scalar][:c.wp_queues]
                        wp_fixed = None
                        for n in range(2):
                            for kk in range(32):
                                if c.dbg_fixed_wp:
                                    # timing-only: one resident wp tile
                                    if wp_fixed is None:
                                        wp_fixed = sbD.tile(
                                            [128, 512], c.dt_mlp,
                                            name="wp_fixed", tag="wp", bufs=5)
                                        nc.sync.dma_start(
                                            wp_fixed[:], wpoT_in[0:128, 0:512])
                                    wp = wp_fixed
                                else:
                                    wp = sbD.tile([128, 512], c.dt_mlp,
                                                  name=f"wp_{n}_{kk}", tag="wp",
                                                  bufs=5)
                                    wp_engs[(n * 32 + kk) % len(wp_engs)].dma_start(
                                        wp[:], wpoT_in[128 * kk:128 * (kk + 1),
                                                       n * 512:(n + 1) * 512])
                                for m in range(4):
                                    nc.tensor.matmul(
                                        ppo_t[m][:],
                                        gT[kk][:, m * 128:(m + 1) * 128],
                                        wp[:], start=(kk == 0),
                                        stop=(kk == 31 and skip))
                                if kk == 31 and not skip:
                                    for m in range(4):
                                        nc.tensor.matmul(
                                            ppo_t[m][:], ones128[:],
                                            bpo_r[:, n * 512:(n + 1) * 512],
                                            start=False, stop=True)
                            for m in range(4):
                                nc.vector.tensor_add(
                                    outb[m][:, n * 512:(n + 1) * 512],
                                    ppo_t[m][:], x2[m][:, n * 512:(n + 1) * 512])
                                eng = nc.sync if m % 2 == 0 else nc.gpsimd
                                eng.dma_start(
                                    out_ext[128 * m:128 * (m + 1),
                                            n * 512:(n + 1) * 512],
                                    outb[m][:, n * 512:(n + 1) * 512])
    nc.compile()
    return nc


def emit_tanh_gelu(nc, small, out_t, pf, bias_ap, m):
    """Exact GPT-2 gelu_new: 0.5x(1+tanh(sqrt(2/pi)(x+0.044715x^3)))."""
    xf = small.tile([128, 512], F32, name=f"g_x_{m}", tag="g_x")
    if bias_ap is None:
        nc.scalar.copy(xf[:], pf[:])
    else:
        nc.scalar.activation(xf[:], pf[:], AF.Identity, bias=bias_ap)
    s = small.tile([128, 512], F32, name=f"g_s_{m}", tag="g_s")
    nc.scalar.activation(s[:], xf[:], AF.Square)
    nc.vector.tensor_scalar(s[:], s[:], 0.044715, 1.0,
                            op0=mybir.AluOpType.mult, op1=mybir.AluOpType.add)
    nc.vector.tensor_mul(s[:], s[:], xf[:])
    th = small.tile([128, 512], F32, name=f"g_t_{m}", tag="g_t")
    nc.scalar.activation(th[:], s[:], AF.Tanh, scale=0.7978845608028654)
    nc.vector.tensor_mul(th[:], th[:], xf[:])
    nc.vector.tensor_add(th[:], th[:], xf[:])
    nc.scalar.activation(out_t[:], th[:], AF.Copy, scale=0.5)


# ======================= host side =======================

def prep_inputs(core, inputs, cfg):
    c = cfg
    x = np.ascontiguousarray(np.asarray(inputs["x"], np.float32).reshape(T, E))
    w_qkv = np.asarray(inputs["w_qkv"], np.float32)
    b_qkv = np.asarray(inputs["b_qkv"], np.float32)
    # reference layout: qkv.reshape(B,S,H,3*HD) -> head h rows are
    # w_qkv[192h:192h+64]=q, [+64:+128]=k, [+128:+192]=v
    hs = [2 * core, 2 * core + 1]
    # fold LN1 gamma/beta into the QKV weights/bias: W(g*xn + b) + b0 =
    # (W*g) xn + (b0 + W b)
    g1 = np.asarray(inputs["ln1_g"], np.float32)
    bt1 = np.asarray(inputs["ln1_b"], np.float32)
    w_qkv = w_qkv * g1[None, :]
    b_qkv = b_qkv + np.asarray(inputs["w_qkv"], np.float32) @ bt1
    wq = np.concatenate([w_qkv[192 * h:192 * h + 64] for h in hs]) * 0.125
    wk = np.concatenate([w_qkv[192 * h + 64:192 * h + 128] for h in hs])
    wv = np.concatenate([w_qkv[192 * h + 128:192 * h + 192] for h in hs])
    wqkvT = np.concatenate([wq, wk, wv], axis=0).T.copy()
    bq = np.concatenate([b_qkv[192 * h:192 * h + 64] for h in hs]) * 0.125
    bk = np.concatenate([b_qkv[192 * h + 64:192 * h + 128] for h in hs])
    bv = np.concatenate([b_qkv[192 * h + 128:192 * h + 192] for h in hs])
    g2 = np.asarray(inputs["ln2_g"], np.float32)
    bt2 = np.asarray(inputs["ln2_b"], np.float32)
    w_fc = np.asarray(inputs["w_fc"], np.float32) * g2[None, :]
    b_fc = (np.asarray(inputs["b_fc"], np.float32)
            + np.asarray(inputs["w_fc"], np.float32) @ bt2)
    negtri = np.where(np.arange(128)[:, None] > np.arange(128)[None, :],
                      np.float32(NEG), np.float32(0.0)).astype(np.float32)
    selbc = np.zeros((8, 512), np.float32)
    for k in range(8):
        selbc[k, 64 * k:64 * k + 64] = 1.0
    return {
        "x": x.astype(np_dt(c.dt_x)),
        "x_own": (x[core * TC:(core + 1) * TC]
                  + np.asarray(inputs["b_ao"], np.float32)[None, :]).copy(),
        "wqkvT": wqkvT.astype(np_dt(c.dt_qkv)),
        "bqkv": np.stack([bq, bk, bv], axis=1).copy(),
        "waoT": np.asarray(inputs["w_ao"], np.float32).T.copy().astype(np_dt(c.dt_ao)),
        "bao": np.asarray(inputs["b_ao"], np.float32).reshape(1, E).copy(),
        "wfcT": w_fc.T.copy().astype(np_dt(c.dt_mlp)),
        "bfc": b_fc.reshape(32, 128).T.copy(),
        "wpoT": np.asarray(inputs["w_po"], np.float32).T.copy().astype(np_dt(c.dt_mlp)),
        "bpo": np.asarray(inputs["b_po"], np.float32).reshape(1, E).copy(),
        "ln1g": np.asarray(inputs["ln1_g"], np.float32).reshape(8, 128).T.copy(),
        "ln1b": np.asarray(inputs["ln1_b"], np.float32).reshape(8, 128).T.copy(),
        "ln2g": np.asarray(inputs["ln2_g"], np.float32).reshape(8, 128).T.copy(),
        "ln2b": np.asarray(inputs["ln2_b"], np.float32).reshape(8, 128).T.copy(),
        "negtri": negtri,
        "selbc": selbc.astype(np_dt(c.dt_ao)),
    }


def check_trivial(inputs):
    z = lambda a: bool(np.all(np.asarray(a) == 0))
    o = lambda a: bool(np.all(np.asarray(a) == 1))
    return (z(inputs["b_qkv"]) and z(inputs["b_ao"]) and z(inputs["b_fc"])
            and z(inputs["b_po"]) and z(inputs["ln1_b"])
            and z(inputs["ln2_b"]))


_prog_cache = {}


def get_program(cfg):
    if cfg.key not in _prog_cache:
        _prog_cache[cfg.key] = build_program(cfg)
    return _prog_cache[cfg.key]


def run_block(inputs, cfg=None):
    from concourse.bass_utils import run_bass_kernel_spmd
    if cfg is None:
        cfg = Cfg(skip_trivial=check_trivial(inputs))
    nc = get_program(cfg)
    in_maps = [prep_inputs(cc, inputs, cfg) for cc in range(NC)]
    res = run_bass_kernel_spmd(nc, in_maps, list(range(NC)))
    out = np.concatenate([res.results[cc]["out_own"] for cc in range(NC)], axis=0)
    return out.reshape(B, S, E)


def kernel(**inputs):
    """Full-input entry point: takes the problem's full tensors, returns [B,S,E]."""
    return np.asarray(run_block(inputs), np.float32)

